# revision 13
# baseline (speedup 1.0000x reference)
"""Trainium2 Bass kernel for nn_Block_en_49469433315543 (involution block).

Computation (see reference):
  z = softplus(involution(x))          involution: per-pixel 3x3 dynamic kernel
  y = softplus(conv2d_3x3(z) + b_conv2)
with the per-pixel kernel = w_span @ relu(BN(w_reduce @ x)) + b_span, where BN
uses batch statistics over all 8 samples (requires a cross-core AllReduce).

Sharding: data-parallel over batch, one sample per NeuronCore (8 cores).
BN statistics via an augmented Gram matrix S = sum([x;1][x;1]^T) per core.

v3 structure (vs the serial baseline):
  - only xh_0 + x_cm are shipped; the h+-1 shifted copies are derived with
    partition-offset SBUF->SBUF DMAs (halves input DRAM traffic).
  - raw r = W@x computed and drained to fp16 DURING the AllReduce; the BN
    affine+relu applies after stats land.  r/rn are w-major [c, w, h] so the
    kern matmuls get contiguous stationary operands (44ns vs 122ns each).
  - involution MACs run at full width per w-half (big DVE ops amortize the
    per-op drain); softplus+scatter chunk by c-block so the h->c DRAM bounce
    of z pipelines under the MACs and conv2's first w-half starts while the
    second is still MAC-ing.  exp/ln batched to stop activation-table thrash.
  - z bounce DMAs round-robin over all five engines' DGE rings.
  - conv2 reorders taps outer / row-subtiles inner so each weight loads once
    per chunk; y streams out per chunk as two DRAM halves (long descriptors).
"""
import sys

for _p in ("/opt/trn_rl_repo", "/root/.axon_site/_ro/trn_rl_repo"):
    if _p not in sys.path:
        sys.path.insert(0, _p)

import numpy as np

import concourse.bacc as bacc
import concourse.tile as tile
from concourse import mybir
from concourse.bass_utils import run_bass_kernel_spmd

C, H, W = 64, 128, 128
HW = H * W
N_CORES = 8
NTOT = N_CORES * HW
BN_EPS = 1e-5
WP = 132          # padded w width in h-major x layout (2 zeros each side)
ZP = 130          # padded side of z in channel-major layout
WH = W // 2       # output half width
F16 = mybir.dt.float16
F32 = mybir.dt.float32

# involution w-ranges: first range covers the halo col needed by conv2 half 0
INV_W = [(0, 66), (66, 128)]
CB = 32           # z-scatter c-block size
WB = 16           # rn/kern w-block size

_CACHE = {}


def _build():
    nc = bacc.Bacc()
    dp = nc.declare_dram_parameter
    x_cm = dp("x_cm", [C, HW], F16, isOutput=False)
    xh_0 = dp("xh_0", [H, C + 1, WP], F16, isOutput=False)
    zrow = dp("zrow", [1, C * WP], F16, isOutput=False)
    wrT = dp("wrT", [C, C], F16, isOutput=False)       # w_reduce^T [c, o]
    wrow = dp("wrow", [C, C], F16, isOutput=False)     # w_reduce   [o, c]
    wspanT = dp("wspanT", [C, 9], F16, isOutput=False)
    bspan_bc = dp("bspan_bc", [H, 9], F32, isOutput=False)
    w_pair = [dp(f"wp{i}", [2 * C, C], F16, isOutput=False) for i in range(3)]
    w_sing = [dp(f"ws{i}", [C, C], F16, isOutput=False) for i in range(3)]
    gamma = dp("gamma", [C, 1], F32, isOutput=False)
    beta = dp("beta", [C, 1], F32, isOutput=False)
    bred = dp("bred", [C, 1], F32, isOutput=False)
    bconv = dp("bconv", [C, 1], F32, isOutput=False)
    y_half = [dp(f"y{s}", [C, H * WH], F32, isOutput=True) for s in range(2)]

    AF = mybir.ActivationFunctionType
    OP = mybir.AluOpType

    with tile.TileContext(nc) as tc:
        with (
            tc.tile_pool(name="sbuf", bufs=1) as pool,
            tc.tile_pool(name="rot", bufs=2) as rot,
            tc.tile_pool(name="psum", bufs=2, space="PSUM") as pp,
            tc.tile_pool(name="dram", bufs=1, space="DRAM") as dram,
        ):
            # ---- load inputs --------------------------------------------
            t_xh0 = pool.tile([H, C + 1, WP], F16)
            nc.sync.dma_start(t_xh0[:], xh_0[:])
            t_xcm = pool.tile([C, HW], F16)
            nc.scalar.dma_start(t_xcm[:], x_cm[:])
            t_wrT = pool.tile([C, C], F16)
            t_wrow = pool.tile([C, C], F16)
            t_wspanT = pool.tile([C, 9], F16)
            t_bspan = pool.tile([H, 9], F32)
            nc.gpsimd.dma_start(t_wrT[:], wrT[:])
            nc.gpsimd.dma_start(t_wrow[:], wrow[:])
            nc.gpsimd.dma_start(t_wspanT[:], wspanT[:])
            nc.gpsimd.dma_start(t_bspan[:], bspan_bc[:])
            t_wp = [pool.tile([2 * C, C], F16, name=f"twp{i}", tag=f"wp{i}") for i in range(3)]
            t_ws = [pool.tile([C, C], F16, name=f"tws{i}", tag=f"ws{i}") for i in range(3)]
            for i in range(3):
                nc.gpsimd.dma_start(t_wp[i][:], w_pair[i][:])
                nc.gpsimd.dma_start(t_ws[i][:], w_sing[i][:])
            t_gamma = pool.tile([C, 1], F32)
            t_beta = pool.tile([C, 1], F32)
            t_bred = pool.tile([C, 1], F32)
            t_bconv = pool.tile([C, 1], F32)
            nc.gpsimd.dma_start(t_gamma[:], gamma[:])
            nc.gpsimd.dma_start(t_beta[:], beta[:])
            nc.gpsimd.dma_start(t_bred[:], bred[:])
            nc.gpsimd.dma_start(t_bconv[:], bconv[:])

            # h+-1 shifted x copies via partition-offset SBUF->SBUF DMA
            t_xhm = pool.tile([H, C, WP], F16)
            t_xhp = pool.tile([H, C, WP], F16)
            nc.gpsimd.dma_start(t_xhm[1:H, :, :], t_xh0[0 : H - 1, 0:C, :])
            nc.gpsimd.dma_start(t_xhm[0:1, :, :], zrow[:].rearrange("o (c w) -> o c w", c=C))
            nc.gpsimd.dma_start(t_xhp[0 : H - 1, :, :], t_xh0[1:H, 0:C, :])
            nc.gpsimd.dma_start(t_xhp[H - 1 : H, :, :], zrow[:].rearrange("o (c w) -> o c w", c=C))

            # ---- Gram stats: S_aug = sum over pixels of [x;1][x;1]^T ----
            ps_S = pp.tile([C + 1, C + 1], F32, tag="ps")
            for w in range(2, 2 + W):
                sl = t_xh0[:, 0 : C + 1, w]
                nc.tensor.matmul(
                    ps_S[:], lhsT=sl, rhs=sl, start=(w == 2), stop=(w == 2 + W - 1)
                )
            t_S32 = pool.tile([C + 1, C + 1], F32)
            nc.vector.tensor_copy(out=t_S32[:], in_=ps_S[:])

            # ---- AllReduce of S across the 8 cores ----------------------
            d_sin = dram.tile([C + 1, C + 1], F32)
            d_sout = dram.tile([C + 1, C + 1], F32)
            nc.sync.dma_start(d_sin[:], t_S32[:])
            nc.gpsimd.collective_compute(
                "AllReduce",
                OP.add,
                replica_groups=[list(range(N_CORES))],
                ins=[d_sin.opt()],
                outs=[d_sout.opt()],
            )
            t_G = pool.tile([C + 1, C + 1], F32)
            nc.sync.dma_start(t_G[:], d_sout[:])

            # ---- raw r = W@x (overlaps the AllReduce) -------------------
            # w-major layout: r[c, w, h] so later stages pipeline per w-block
            t_r = pool.tile([C, W, H], F16, tag="big1")
            xcm_v = t_xcm[:].rearrange("c (h w) -> c h w", h=H)
            NWB = W // WB
            for j in range(NWB):
                w0 = j * WB
                ps_r = pp.tile([C, 4, 32, WB], F32, tag="ps")
                for hh in range(4):
                    rhs = xcm_v[:, hh * 32 : (hh + 1) * 32, w0 : w0 + WB]
                    nc.tensor.matmul(ps_r[:, hh, :, :], lhsT=t_wrT[:], rhs=rhs)
                for hh in range(2):
                    # drain transposes (h, w) -> (w, h): r becomes w-major
                    nc.vector.tensor_copy(
                        out=t_r[:, w0 : w0 + WB, hh * 64 : (hh + 1) * 64],
                        in_=ps_r[:, 2 * hh : 2 * hh + 2, :, :].rearrange(
                            "c s h w -> c w (s h)"
                        ),
                    )

            # ---- BN statistics from the Gram matrix ---------------------
            # xbar = G[0:64, 64] / N ; mu = W xbar + b
            t_xbar16 = pool.tile([C, 1], F16)
            nc.vector.tensor_scalar_mul(t_xbar16[:], t_G[0:C, C : C + 1], 1.0 / NTOT)
            ps_mu = pp.tile([C, 1], F32, tag="ps")
            nc.tensor.matmul(ps_mu[:], lhsT=t_wrT[:], rhs=t_xbar16[:])
            t_mu = pool.tile([C, 1], F32)
            nc.vector.tensor_tensor(out=t_mu[:], in0=ps_mu[:], in1=t_bred[:], op=OP.add)
            # T1 = W S/N ; diag = rowsum(T1 * W)
            t_S16 = pool.tile([C, C], F16)
            nc.vector.tensor_scalar_mul(t_S16[:], t_G[0:C, 0:C], 1.0 / NTOT)
            ps_T1 = pp.tile([C, C], F32, tag="ps")
            nc.tensor.matmul(ps_T1[:], lhsT=t_wrT[:], rhs=t_S16[:])
            t_q = pool.tile([C, C], F32)
            nc.vector.tensor_tensor(
                out=t_q[:], in0=ps_T1[:], in1=t_wrow[:], op=OP.mult
            )
            t_diag = pool.tile([C, 1], F32)
            nc.vector.tensor_reduce(
                t_diag[:], t_q[:], axis=mybir.AxisListType.X, op=OP.add
            )
            # E2 = diag + b*(2 mu - b); var = E2 - mu^2
            t_u = pool.tile([C, 1], F32)
            nc.vector.tensor_scalar_mul(t_u[:], t_mu[:], 2.0)
            nc.vector.tensor_tensor(out=t_u[:], in0=t_u[:], in1=t_bred[:], op=OP.subtract)
            nc.vector.tensor_tensor(out=t_u[:], in0=t_u[:], in1=t_bred[:], op=OP.mult)
            t_e2 = pool.tile([C, 1], F32)
            nc.vector.tensor_tensor(out=t_e2[:], in0=t_diag[:], in1=t_u[:], op=OP.add)
            t_mu2 = pool.tile([C, 1], F32)
            nc.vector.tensor_tensor(out=t_mu2[:], in0=t_mu[:], in1=t_mu[:], op=OP.mult)
            t_var = pool.tile([C, 1], F32)
            nc.vector.tensor_tensor(out=t_var[:], in0=t_e2[:], in1=t_mu2[:], op=OP.subtract)
            # rstd = sqrt(1/(var + eps)); a = gamma*rstd; bb = a*(b-mu)+beta
            nc.vector.tensor_scalar_add(t_var[:], t_var[:], BN_EPS)
            t_rvar = pool.tile([C, 1], F32)
            nc.vector.reciprocal(t_rvar[:], t_var[:])
            t_rstd = pool.tile([C, 1], F32)
            nc.scalar.activation(t_rstd[:], t_rvar[:], AF.Sqrt)
            t_a = pool.tile([C, 1], F32)
            nc.vector.tensor_tensor(out=t_a[:], in0=t_gamma[:], in1=t_rstd[:], op=OP.mult)
            t_bb = pool.tile([C, 1], F32)
            nc.vector.tensor_tensor(out=t_bb[:], in0=t_bred[:], in1=t_mu[:], op=OP.subtract)
            nc.vector.tensor_tensor(out=t_bb[:], in0=t_bb[:], in1=t_a[:], op=OP.mult)
            nc.vector.tensor_tensor(out=t_bb[:], in0=t_bb[:], in1=t_beta[:], op=OP.add)

            # ---- per w-block: rn = relu(a*r + bb); kern matmuls ---------
            # kern_h[h, k, w] = sum_c rn[c, w, h] wspanT[c, k] + b_span[k]
            t_rn = pool.tile([C, W, H], F16, tag="t_xcm")  # alias over x_cm
            t_kern = pool.tile([H, 9, W], F16)
            WG = 8
            for j in range(NWB):
                w0 = j * WB
                nc.scalar.activation(
                    t_rn[:, w0 : w0 + WB, :],
                    t_r[:, w0 : w0 + WB, :],
                    AF.Relu,
                    bias=t_bb[:],
                    scale=t_a[:],
                )
                for g in range(WB // WG):
                    wg0 = w0 + g * WG
                    ps_k = pp.tile([H, 9 * WG], F32, tag="ps")
                    for jj in range(WG):
                        lhs = t_rn[:, wg0 + jj, :]  # [64, 128] contiguous
                        nc.tensor.matmul(
                            ps_k[:, jj * 9 : (jj + 1) * 9], lhsT=lhs, rhs=t_wspanT[:]
                        )
                    src = ps_k[:].rearrange("h (j k) -> h k j", k=9)
                    dst = t_kern[:, :, wg0 : wg0 + WG]
                    bias = t_bspan[:].rearrange("h (o k) -> h k o", o=1).broadcast_to(
                        [H, 9, WG]
                    )
                    nc.vector.scalar_tensor_tensor(
                        out=dst, in0=src, scalar=1.0, in1=bias, op0=OP.mult, op1=OP.add
                    )

            # ---- zz conv-input tile: borders zeroed, aliased over r -----
            t_zz = pool.tile([2 * C, ZP * ZP], F16, tag="big1")
            zz_v2 = t_zz[:].rearrange("c (h w) -> c h w", h=ZP)
            nc.gpsimd.memset(zz_v2[:, 0, :], 0.0)
            nc.gpsimd.memset(zz_v2[:, ZP - 1, :], 0.0)
            nc.gpsimd.memset(zz_v2[0:C, 1 : 1 + H, 0:1], 0.0)
            nc.gpsimd.memset(zz_v2[0:C, 1 : 1 + H, ZP - 1 : ZP], 0.0)
            nc.gpsimd.memset(zz_v2[C : 2 * C, 1 : 1 + H, ZP - 2 : ZP], 0.0)
            zz_lo_v = t_zz[0:C, :].rearrange("c (h w) -> c h w", h=ZP)
            zz_hi_v = t_zz[C : 2 * C, :].rearrange("c (h w) -> c h w", h=ZP)

            # ---- involution MAC (DVE, full width per w-half) ------------
            xh_by_dh = {-1: t_xhm, 0: t_xh0, 1: t_xhp}
            eng_rr = [nc.sync, nc.scalar, nc.gpsimd]
            n_dma = 0
            for (w0, w1) in INV_W:
                wl = w1 - w0
                t_acc = rot.tile([H, C, wl], F16, name="acc", tag="acc")
                t_tmp = rot.tile([H, C, wl], F16, name="mactmp", tag="mactmp")
                first = True
                for i in range(3):
                    for jj in range(3):
                        k = i * 3 + jj
                        dh, dw = i - 1, jj - 1
                        xt = xh_by_dh[dh]
                        x_sl = xt[:, 0:C, 2 + dw + w0 : 2 + dw + w1]
                        k_bc = (
                            t_kern[:, k, w0:w1]
                            .rearrange("h (o w) -> h o w", o=1)
                            .broadcast_to([H, C, wl])
                        )
                        if first:
                            nc.vector.tensor_tensor(
                                out=t_acc[:], in0=x_sl, in1=k_bc, op=OP.mult
                            )
                            first = False
                        else:
                            nc.vector.tensor_tensor(
                                out=t_tmp[:], in0=x_sl, in1=k_bc, op=OP.mult
                            )
                            nc.vector.tensor_tensor(
                                out=t_acc[:], in0=t_acc[:], in1=t_tmp[:], op=OP.add
                            )
                # softplus = ln(1+exp(.)): exp batch then ln batch per c-block
                # (batching keeps the scalar engine on one act table at a time)
                t_es = []
                for cb in range(C // CB):
                    c0 = cb * CB
                    t_e = rot.tile([H, CB, wl], F16, name=f"spe{cb}", tag=f"spe{cb}")
                    nc.scalar.activation(t_e[:], t_acc[:, c0 : c0 + CB, :], AF.Exp)
                    t_es.append(t_e)
                for cb in range(C // CB):
                    c0 = cb * CB
                    t_zh = rot.tile([H, CB, wl], F16, name="zh", tag="zh")
                    nc.scalar.activation(t_zh[:], t_es[cb][:], AF.Ln, bias=1.0)
                    # h-major -> c-major partition swap via a DRAM bounce,
                    # DMAs spread round-robin over the five DGE rings
                    d_zc = dram.tile([CB, H, wl], F16, name=f"dz_{w0}_{c0}")
                    eng_rr[n_dma % 3].dma_start(
                        d_zc[:].rearrange("c h w -> h c w"), t_zh[:]
                    )
                    eng_rr[(n_dma + 1) % 3].dma_start(
                        zz_lo_v[c0 : c0 + CB, 1 : 1 + H, 1 + w0 : 1 + w1], d_zc[:]
                    )
                    # hi copy = lo shifted one col, via partition-offset
                    # SBUF->SBUF (cheaper than a second DRAM read)
                    eng_rr[(n_dma + 2) % 3].dma_start(
                        zz_hi_v[c0 : c0 + CB, 1 : 1 + H, w0:w1],
                        zz_lo_v[c0 : c0 + CB, 1 : 1 + H, 1 + w0 : 1 + w1],
                    )
                    n_dma += 1

            # ---- conv2 per w-half (6 matmuls per 8-row sub) -------------
            CROWS = 16  # output rows per psum chunk
            NSUB = CROWS // 8
            for s in range(2):
                wofs = s * WH
                yv = y_half[s][:].rearrange("c (h w) -> c h w", w=WH)
                t_eyh = pool.tile([C, 8, CROWS * WH], F16, name=f"eyh{s}", tag="eyh")
                for ch in range(H // CROWS):
                    ps_y = pp.tile([C, CROWS * WH], F32, tag="ps")
                    # taps outer, row-subtiles inner: one weight load per tap
                    for t in range(6):
                        if t < 3:
                            i = t
                            lhsT_w = t_wp[i][:]
                            part = 2 * C
                            cofs = 0
                        else:
                            i = t - 3
                            lhsT_w = t_ws[i][:]
                            part = C
                            cofs = 2
                        for sub in range(NSUB):
                            h0 = ch * CROWS + sub * 8
                            src2 = zz_v2[
                                0:part, h0 + i : h0 + i + 8,
                                cofs + wofs : cofs + wofs + WH,
                            ]
                            nc.tensor.matmul(
                                ps_y[:, sub * 8 * WH : (sub + 1) * 8 * WH],
                                lhsT=lhsT_w,
                                rhs=src2,
                                start=(t == 0),
                                stop=(t == 5),
                            )
                    nc.scalar.activation(
                        t_eyh[:, ch, :], ps_y[:], AF.Exp, bias=t_bconv[:]
                    )
                for ch in range(H // CROWS):
                    t_y = rot.tile([C, CROWS * WH], F32, tag="yc")
                    nc.scalar.activation(t_y[:], t_eyh[:, ch, :], AF.Ln, bias=1.0)
                    eng_y = nc.sync if s == 0 else nc.scalar
                    eng_y.dma_start(
                        yv[:, ch * CROWS : (ch + 1) * CROWS, :], t_y[:]
                    )

    nc.compile()
    return nc


def _prep_core_inputs(xs, w_reduce, b_reduce, bn_gamma, bn_beta, w_span, b_span,
                      w_conv2, b_conv2):
    """Host-side layout prep for one core's sample xs [C, H, W] fp32."""
    xhw = xs.transpose(1, 0, 2)  # [h, c, w]
    xh_0 = np.zeros((H, C + 1, WP), np.float16)
    xh_0[:, 0:C, 2 : 2 + W] = xhw
    xh_0[:, C, 2 : 2 + W] = 1.0
    w_pair = []
    w_sing = []
    for i in range(3):
        wp = np.concatenate(
            [w_conv2[:, :, i, 0].T, w_conv2[:, :, i, 1].T], axis=0
        ).astype(np.float16)
        w_pair.append(np.ascontiguousarray(wp))
        w_sing.append(np.ascontiguousarray(w_conv2[:, :, i, 2].T).astype(np.float16))
    m = {
        "x_cm": xs.reshape(C, HW).astype(np.float16),
        "xh_0": xh_0,
        "zrow": np.zeros((1, C * WP), np.float16),
        "wrT": np.ascontiguousarray(w_reduce.T).astype(np.float16),
        "wrow": np.ascontiguousarray(w_reduce).astype(np.float16),
        "wspanT": np.ascontiguousarray(w_span.T).astype(np.float16),
        "bspan_bc": np.tile(b_span.astype(np.float32)[None, :], (H, 1)),
        "gamma": bn_gamma.astype(np.float32).reshape(C, 1),
        "beta": bn_beta.astype(np.float32).reshape(C, 1),
        "bred": b_reduce.astype(np.float32).reshape(C, 1),
        "bconv": b_conv2.astype(np.float32).reshape(C, 1),
    }
    for i in range(3):
        m[f"wp{i}"] = w_pair[i]
        m[f"ws{i}"] = w_sing[i]
    return m


def kernel(x, w_reduce, b_reduce, bn_gamma, bn_beta, w_span, b_span, w_conv2,
           b_conv2):
    x = np.asarray(x, np.float32)
    if "nc" not in _CACHE:
        _CACHE["nc"] = _build()
    nc = _CACHE["nc"]
    in_maps = [
        _prep_core_inputs(
            x[b], np.asarray(w_reduce, np.float32), np.asarray(b_reduce, np.float32),
            np.asarray(bn_gamma, np.float32), np.asarray(bn_beta, np.float32),
            np.asarray(w_span, np.float32), np.asarray(b_span, np.float32),
            np.asarray(w_conv2, np.float32), np.asarray(b_conv2, np.float32),
        )
        for b in range(N_CORES)
    ]
    res = run_bass_kernel_spmd(nc, in_maps, core_ids=list(range(N_CORES)))
    out = np.empty((N_CORES, C, H, W), np.float32)
    for b in range(N_CORES):
        out[b, :, :, 0:WH] = res.results[b]["y0"].reshape(C, H, WH)
        out[b, :, :, WH:W] = res.results[b]["y1"].reshape(C, H, WH)
    return out.astype(np.float32)
